# revision 1
# baseline (speedup 1.0000x reference)
"""Trainium2 Bass kernel for a GQA transformer block (parallel-residual).

Reference computation (B=2, T=2048, C=2048, 16 heads / 4 query groups,
head_size=128, rope_n_elem=32, ffn=4C):
    qkv = LN1(x) @ w_qkv + b_qkv        (LN scale/bias folded into w/b host-side)
    q,k,v split per query group; RoPE on first 32 channels of q,k
    y   = causal_attention(q, k, v)
    h   = y @ w_proj + b_proj
    mlp = gelu(LN2(x) @ w_fc1 + b_fc1) @ w_fc2 + b_fc2
    out = mlp + h + x

Sharding: 8 cores = 2-way batch-parallel x 4-way tensor-parallel over query
groups.  Core c handles batch b=c//4, group g=c%4.  Each core emits two
partial outputs (mlp partial and h partial, both without output biases);
the host sums them plus x + biases.  No on-device collectives.

Design notes:
- The host passes x TRANSPOSED (feature-major, bf16).  LN stats (mean /
  variance per token) are computed with all-ones stationary matmuls
  (partition reduction on PE), producing partition-replicated [128,512]
  stat tiles; xhatT is then computed elementwise.  No PE transposes.
- fp8 (e4m3) DoubleRow matmuls (2x PE throughput) where numerics allow:
  attention PV + denominator for t>=512, out-projection for t>=512, and
  the upper half of the fc2 contraction.  Early tokens (t<512) stay
  bf16: with a small softmax support the fp8 quantization of probs/v/y
  creates outlier errors there.
- exp pieces are scaled by 1/16 (bias=-ln16) to fit fp8's 240 max; the
  softmax normalization cancels the scale exactly.
- Causal masking is trimmed: for a diagonal piece j only the 128-wide
  boundary block needs a mask multiply; columns below are memset to 0
  and columns above are exp'd directly.
- fc2 weights are scaled x64 (both halves) to share one PSUM group;
  evictions descale by 1/64.
- h is added on the host: fc2 writes the mlp partial to `out`, proj
  writes the h partial to `hout`.
- The first fc1 m-blocks are emitted interleaved into the attention
  group loop to fill PE gaps (the exp/DVE chain otherwise leaves the
  tensor engine idle long enough for the HAM clock throttle to drop it
  to half clock).
"""

import sys

sys.path.insert(0, "/opt/trn_rl_repo")

import math
import numpy as np
import ml_dtypes

import concourse.bass as bass
import concourse.mybir as mybir
import concourse.tile as tile
from concourse.bass_utils import run_bass_kernel_spmd

F32 = mybir.dt.float32
BF16 = mybir.dt.bfloat16
F8 = mybir.dt.float8e4
AF = mybir.ActivationFunctionType
ALU = mybir.AluOpType
DR = mybir.MatmulPerfMode.DoubleRow
BF16NP = ml_dtypes.bfloat16
F8NP = ml_dtypes.float8_e4m3

P = 128
T = 2048
C = 2048
D = 128
NT = T // P          # token tiles
NK = C // P          # contraction tiles over C
QH = 4               # query heads per group
GCOLS = (QH + 2) * D  # 768 qkv columns per group
FFN_S = 2048         # ffn shard per core (8192/4)
NF = FFN_S // P
NF_B = 8             # fc2 k-tiles computed in bf16 (0..7); 8..15 in fp8 DR
N_EARLY = 6          # fc1 m-blocks interleaved into the attention loop
LN_EPS = 1e-5
WS = 64.0            # fp8 weight scale (and bf16-fc2 scale for psum sharing)
EB = -math.log(16.0)  # exp bias: pieces = exp(score - ln 16)

_CACHED_NC = None


def _split_sync_waits(nc, limit=1):
    """This walrus build rejects instructions carrying more than one sem wait
    (setupSyncWait 'Too many sync wait commands'); move excess waits onto
    preceding NoOps on the same engine."""
    for f in nc.m.functions:
        for blk in f.blocks:
            new_list = []
            for inst in blk.instructions:
                si = inst.sync_info
                if si is not None and si.on_wait is not None and len(si.on_wait) > limit:
                    waits = list(si.on_wait)
                    head, rest = waits[:limit], waits[limit:]
                    k = 0
                    while rest:
                        chunk, rest = rest[:limit], rest[limit:]
                        new_list.append(
                            mybir.InstNoOp(
                                name=f"{inst.name}-ws{k}",
                                sync_info=mybir.SyncInfo(on_wait=chunk, on_update=[]),
                                bass_nofuse=True,
                                engine=inst.engine,
                            )
                        )
                        k += 1
                    inst.sync_info = mybir.SyncInfo(
                        on_wait=head, on_update=list(si.on_update or [])
                    )
                new_list.append(inst)
            blk.instructions[:] = new_list


def build_program():
    nc = bass.Bass()
    with tile.TileContext(nc) as tc:
        dram_cm = tc.tile_pool(name="dram", bufs=1, space="DRAM")
        dram = dram_cm.__enter__()
        xT_in = dram.tile([C, T], BF16, kind="ExternalInput", name="xT", uniquify=False)
        wqkv_in = dram.tile([C, GCOLS], BF16, kind="ExternalInput", name="wqkv", uniquify=False)
        bqkvT_in = dram.tile([P, 6], F32, kind="ExternalInput", name="bqkvT", uniquify=False)
        cosT_in = dram.tile([32, T], BF16, kind="ExternalInput", name="cosT", uniquify=False)
        sinT_in = dram.tile([32, T], BF16, kind="ExternalInput", name="sinT", uniquify=False)
        wprojb_in = dram.tile([QH * D, C], BF16, kind="ExternalInput", name="wprojb", uniquify=False)
        wproj8_in = dram.tile([P, QH, C], F8, kind="ExternalInput", name="wproj8", uniquify=False)
        wfc1_in = dram.tile([C, FFN_S], BF16, kind="ExternalInput", name="wfc1", uniquify=False)
        negSq_in = dram.tile([1, GCOLS], BF16, kind="ExternalInput", name="negSq", uniquify=False)
        negSf1_in = dram.tile([1, FFN_S], BF16, kind="ExternalInput", name="negSf1", uniquify=False)
        bfc1T_in = dram.tile([P, NF], F32, kind="ExternalInput", name="bfc1T", uniquify=False)
        wfc2b_in = dram.tile([P, NF_B, C], BF16, kind="ExternalInput", name="wfc2b", uniquify=False)
        wfc28_in = dram.tile([P, NF - NF_B, C], F8, kind="ExternalInput", name="wfc28", uniquify=False)
        out_d = dram.tile([T, C], BF16, kind="ExternalOutput", name="out", uniquify=False)
        hout_d = dram.tile([T, C], BF16, kind="ExternalOutput", name="hout", uniquify=False)

        # ---- persistent pools ----
        const_cm = tc.tile_pool(name="const", bufs=1)
        const = const_cm.__enter__()
        # 128x128 lower-triangular boundary mask: 1 where t - s >= 0
        tri = const.tile([P, P], BF16, tag="tri")
        nc.gpsimd.memset(tri[:], 1.0)
        nc.gpsimd.affine_select(
            out=tri[:], in_=tri[:], compare_op=ALU.is_ge, fill=0.0,
            base=0, pattern=[[1, P]], channel_multiplier=-1)
        ones_bf = const.tile([P, P], BF16, tag="ones_bf")
        nc.vector.memset(ones_bf[:], 1.0)
        ones8 = const.tile([P, 2, P], F8, tag="ones8")
        nc.vector.memset(ones8[:], 1.0)
        bqkvT = const.tile([P, 6], F32, tag="bqkvT")
        bfc1T = const.tile([P, NF], F32, tag="bfc1T")
        eps_t = const.tile([P, 1], F32, tag="eps")
        nc.vector.memset(eps_t[:], LN_EPS)
        eb_t = const.tile([P, 1], F32, tag="eb")
        nc.vector.memset(eb_t[:], EB)
        negSq = const.tile([1, GCOLS], BF16, tag="negSq")
        negSf1 = const.tile([1, FFN_S], BF16, tag="negSf1")
        mur_all = const.tile([1, T], BF16, tag="mur")

        def emit_const_dmas():
            nc.sync.dma_start(out=bqkvT[:], in_=bqkvT_in[:])
            nc.sync.dma_start(out=bfc1T[:], in_=bfc1T_in[:])
            nc.sync.dma_start(out=negSq[:], in_=negSq_in[:])
            nc.sync.dma_start(out=negSf1[:], in_=negSf1_in[:])

        xhatT_cm = tc.tile_pool(name="xhatT", bufs=NK)
        xhatT_pool = xhatT_cm.__enter__()
        xhatT = [xhatT_pool.tile([P, T], BF16, tag="xhatT", name=f"xhatT{i}") for i in range(NK)]

        # fc1 weight-block stream (lives through attention + fc1)
        wblk_cm = tc.tile_pool(name="wblk", bufs=1)
        wblk_pool = wblk_cm.__enter__()

        # shared matmul-accumulator psum pools
        psMM_cm = tc.tile_pool(name="psMM", bufs=4, space="PSUM")
        psMM = psMM_cm.__enter__()
        psY_cm = tc.tile_pool(name="psY", bufs=2, space="PSUM")
        psY = psY_cm.__enter__()
        psD_cm = tc.tile_pool(name="psD", bufs=2, space="PSUM")
        psD = psD_cm.__enter__()

        # ========== Stage A+B: LN stats + xhatT, then QKV ==================
        qkvT_cm = tc.tile_pool(name="qkvT", bufs=5)
        qkvT_pool = qkvT_cm.__enter__()
        qkvT = [qkvT_pool.tile([P, T], BF16, tag="qkvT", name=f"qkvT{i}") for i in range(5)]

        def emit_rope(m, nch, pool, ct, st):
            # rope = x*cos + rot16(x)*sinT; sinT sign-folded by the host;
            # rot16 via partition-shifting SBUF->SBUF DMAs.  ct/st are the
            # [32,512] cos/sin slices for this chunk.
            ch = slice(nch * 512, (nch + 1) * 512)
            rot = pool.tile([32, 512], BF16, tag="rot", bufs=3, name=f"rot{m}_{nch}")
            nc.sync.dma_start(out=rot[0:16, :], in_=qkvT[m][16:32, ch])
            nc.sync.dma_start(out=rot[16:32, :], in_=qkvT[m][0:16, ch])
            t_cos = pool.tile([32, 512], BF16, tag="t_cos", bufs=3, name=f"tc{m}_{nch}")
            nc.vector.tensor_tensor(out=t_cos[:], in0=qkvT[m][0:32, ch],
                                    in1=ct, op=ALU.mult)
            nc.gpsimd.tensor_tensor(out=rot[:], in0=rot[:],
                                    in1=st, op=ALU.mult)
            nc.gpsimd.tensor_tensor(out=qkvT[m][0:32, ch], in0=t_cos[:],
                                    in1=rot[:], op=ALU.add)
        vtok_cm = tc.tile_pool(name="vtok", bufs=1)
        vtok_pool = vtok_cm.__enter__()
        v_tok8 = vtok_pool.tile([P, NK, P], F8, tag="vtok8")
        vb = vtok_pool.tile([P, 4, P], BF16, tag="vb")

        wqkv_cm = tc.tile_pool(name="wqkv", bufs=NK)
        wqkv_pool = wqkv_cm.__enter__()
        wqkv = []


        with tc.tile_pool(name="xio", bufs=16) as xio, \
             tc.tile_pool(name="stat", bufs=2) as stat:

            cos0 = xio.tile([32, 512], BF16, tag="cos0", bufs=1, name="cos0")
            sin0 = xio.tile([32, 512], BF16, tag="sin0", bufs=1, name="sin0")

            # warmup burst: keep PE busy through the first x DMAs so the HAM
            # clock gate reaches 8/8 before the real matmuls start
            wps = psMM.tile([P, 512], F32, tag="mm", name="warm")
            for w in range(40):
                nc.tensor.matmul(wps[:, 0:P], lhsT=ones_bf[:], rhs=ones_bf[:],
                                 start=(w == 0), stop=(w == 39))

            def emit_stats(nch):
                ch = slice(nch * 512, (nch + 1) * 512)
                xq = []
                for k in range(NK):
                    xt = xio.tile([P, 512], BF16, tag="xq", name=f"xq{nch}_{k}", bufs=32)
                    nc.sync.dma_start(out=xt[:], in_=xT_in[k * P:(k + 1) * P, ch])
                    xq.append(xt)
                if nch == 0:
                    # const + weight DMAs after the first x tiles win the race
                    emit_const_dmas()
                    nc.sync.dma_start(out=cos0[:], in_=cosT_in[:, 0:512])
                    nc.sync.dma_start(out=sin0[:], in_=sinT_in[:, 0:512])
                    for k in range(NK):
                        wt = wqkv_pool.tile([P, GCOLS], BF16, tag="wqkv", name=f"wqkv{k}")
                        nc.sync.dma_start(out=wt[:], in_=wqkv_in[k * P:(k + 1) * P, :])
                        wqkv.append(wt)
                xsq = []
                for k in range(NK):
                    sq = xio.tile([P, 512], BF16, tag="xsq", name=f"xsq{nch}_{k}", bufs=10)
                    nc.scalar.activation(out=sq[:], in_=xq[k][:], func=AF.Square,
                                         bias=0.0, scale=1.0)
                    xsq.append(sq)
                psA = psY.tile([P, 512], F32, tag="psy")
                for k in range(NK):
                    nc.tensor.matmul(psA[:], lhsT=ones_bf[:], rhs=xq[k][:],
                                     start=(k == 0), stop=(k == NK - 1))
                psB = psD.tile([P, 512], F32, tag="psd")
                for k in range(NK):
                    nc.tensor.matmul(psB[:], lhsT=ones_bf[:], rhs=xsq[k][:],
                                     start=(k == 0), stop=(k == NK - 1))
                mean = stat.tile([P, 512], F32, tag="mean", bufs=1)
                nc.vector.tensor_scalar(out=mean[:], in0=psA[:], scalar1=1.0 / C,
                                        scalar2=None, op0=ALU.mult)
                var = stat.tile([P, 512], F32, tag="var", bufs=1)
                msq = stat.tile([P, 512], F32, tag="msq", bufs=1)
                nc.gpsimd.tensor_tensor(out=msq[:], in0=mean[:], in1=mean[:], op=ALU.mult)
                nc.vector.tensor_scalar(out=var[:], in0=psB[:], scalar1=1.0 / C,
                                        scalar2=None, op0=ALU.mult)
                nc.gpsimd.tensor_tensor(out=var[:], in0=var[:], in1=msq[:], op=ALU.subtract)
                rstd = stat.tile([P, 512], F32, tag="rstd")
                nc.scalar.activation(out=rstd[:], in_=var[:], func=AF.Sqrt,
                                     bias=eps_t[:], scale=1.0)
                nc.vector.reciprocal(rstd[:], rstd[:])
                # mur row: mean*rstd for this chunk (consumed by the K=1
                # rank-1 LN correction folded into QKV/fc1 matmuls)
                nc.gpsimd.tensor_tensor(out=mur_all[0:1, ch], in0=mean[0:1, :],
                                        in1=rstd[0:1, :], op=ALU.mult)
                # xhatT holds x*rstd (half-normalized); the -mu*rstd part is
                # applied as a rank-1 matmul correction
                for k in range(NK):
                    eng = nc.vector if k % 2 == 0 else nc.gpsimd
                    eng.tensor_tensor(out=xhatT[k][:, ch], in0=xq[k][:], in1=rstd[:],
                                      op=ALU.mult)

            def emit_qkv(nch):
                ch = slice(nch * 512, (nch + 1) * 512)
                for m in (4, 0, 1, 2, 3, 5):
                    if m == 5:
                        # v computed token-major; bias folded into the host-side
                        # output bias (softmax weights sum to 1).
                        for ti in range(4 * nch, 4 * nch + 4):
                            pb = psMM.tile([P, 512], F32, tag="mm")
                            for k in range(NK):
                                nc.tensor.matmul(pb[:, 0:P],
                                                 lhsT=xhatT[k][:, ti * P:(ti + 1) * P],
                                                 rhs=wqkv[k][:, 5 * P:6 * P],
                                                 start=(k == 0), stop=False)
                            nc.tensor.matmul(pb[:, 0:P],
                                             lhsT=mur_all[0:1, ti * P:(ti + 1) * P],
                                             rhs=negSq[0:1, 5 * P:6 * P],
                                             start=False, stop=True)
                            nc.scalar.activation(out=v_tok8[:, ti, :], in_=pb[:, 0:P],
                                                 func=AF.Identity, bias=0.0, scale=1.0)
                            if ti < 4:
                                nc.scalar.copy(vb[:, ti, :], pb[:, 0:P])
                        continue
                    pb = psMM.tile([P, 512], F32, tag="mm")
                    for k in range(NK):
                        nc.tensor.matmul(pb[:], lhsT=wqkv[k][:, m * P:(m + 1) * P],
                                         rhs=xhatT[k][:, ch],
                                         start=(k == 0), stop=False)
                    nc.tensor.matmul(pb[:], lhsT=negSq[0:1, m * P:(m + 1) * P],
                                     rhs=mur_all[0:1, ch], start=False, stop=True)
                    nc.scalar.activation(out=qkvT[m][:, ch],
                                         in_=pb[:], func=AF.Identity,
                                         bias=bqkvT[:, m:m + 1], scale=1.0)
                    if nch == 0:
                        emit_rope(m, nch, xio, cos0[:], sin0[:])

            emit_stats(0)
            emit_stats(1)
            emit_stats(2)
            emit_qkv(0)
            emit_stats(3)
            emit_qkv(1)
            emit_qkv(2)
            emit_qkv(3)

        wqkv_cm.__exit__(None, None, None)

        # ================= Stage C: causal attention ======================
        # (s, t) score layout; fc1 early blocks interleaved to keep PE warm.
        uTa_cm = tc.tile_pool(name="uTa", bufs=1, side="right")
        uTa_pool = uTa_cm.__enter__()
        uT8 = [uTa_pool.tile([P, NF - NF_B, T], F8, tag="uT8", bufs=1, name="uT8")]

        yG_cm = tc.tile_pool(name="yG", bufs=1, side="right")
        yG_pool = yG_cm.__enter__()
        yGb = [yG_pool.tile([P, 512], BF16, tag="yGb", name=f"yGb{i}", bufs=QH) for i in range(QH)]
        yG8 = yG_pool.tile([P, QH, T], F8, tag="yG8", bufs=1)

        wproj_cm = tc.tile_pool(name="wproj", bufs=1)
        wproj_pool = wproj_cm.__enter__()
        hsb_cm = tc.tile_pool(name="hsb", bufs=2)
        hsb_pool = hsb_cm.__enter__()
        wprojb = []
        for k in range(QH):
            wt = wproj_pool.tile([P, C], BF16, tag="wprojb", name=f"wprojb{k}", bufs=QH)
            nc.sync.dma_start(out=wt[:], in_=wprojb_in[k * P:(k + 1) * P, :])
            wprojb.append(wt)
        wproj8 = wproj_pool.tile([P, QH, C], F8, tag="wproj8", bufs=1)
        nc.sync.dma_start(out=wproj8[:], in_=wproj8_in[:])

        wfc1_r = wfc1_in[:].rearrange("(kk p) m -> p kk m", p=P)
        FC1_ORDER = list(range(NF_B, NF)) + list(range(NF_B))
        fc1_dma_next = [0]
        fc1_next = [0]
        from collections import deque
        wb_q = deque()
        uTb = []

        def fc1_dma():
            i = fc1_dma_next[0]
            if i >= NF:
                return
            fc1_dma_next[0] = i + 1
            m = FC1_ORDER[i]
            wb = wblk_pool.tile([P, NK, P], BF16, tag="wb", bufs=2, name=f"wb{m}")
            nc.sync.dma_start(out=wb[:], in_=wfc1_r[:, :, m * P:(m + 1) * P])
            wb_q.append(wb)

        def emit_fc1_block(ps_pools=None):
            i = fc1_next[0]
            if i >= NF:
                return
            fc1_next[0] = i + 1
            m = FC1_ORDER[i]
            if not wb_q:
                fc1_dma()
            wb = wb_q.popleft()
            fc1_dma()
            for nch in range(4):
                if ps_pools is None:
                    pool, ptag = psMM, "mm"
                else:
                    pool, ptag = ps_pools[nch]
                pe_ = pool.tile([P, 512], F32, tag=ptag)
                for k in range(NK):
                    nc.tensor.matmul(pe_[:], lhsT=wb[:, k, :],
                                     rhs=xhatT[k][:, nch * 512:(nch + 1) * 512],
                                     start=(k == 0), stop=False)
                nc.tensor.matmul(pe_[:], lhsT=negSf1[0:1, m * P:(m + 1) * P],
                                 rhs=mur_all[0:1, nch * 512:(nch + 1) * 512],
                                 start=False, stop=True)
                if m < NF_B:
                    dst = uTb[m][:, nch * 512:(nch + 1) * 512]
                else:
                    dst = uT8[0][:, m - NF_B, nch * 512:(nch + 1) * 512]
                nc.scalar.activation(out=dst, in_=pe_[:], func=AF.Gelu,
                                     bias=bfc1T[:, m:m + 1], scale=1.0)

        def emit_D(mt):
            ht = hsb_pool.tile([P, C], BF16, tag="ht")
            for ch in range(4):
                pp = psMM.tile([P, 512], F32, tag="mm")
                if mt < 4:
                    for k in range(QH):
                        nc.tensor.matmul(
                            pp[:],
                            lhsT=yGb[k][:, mt * P:(mt + 1) * P],
                            rhs=wprojb[k][:, ch * 512:(ch + 1) * 512],
                            start=(k == 0), stop=(k == QH - 1))
                    if ch % 2 == 0:
                        nc.vector.tensor_copy(ht[:, ch * 512:(ch + 1) * 512], pp[:])
                    else:
                        nc.scalar.copy(ht[:, ch * 512:(ch + 1) * 512], pp[:])
                else:
                    for j in range(QH // 2):
                        nc.tensor.matmul(
                            pp[:],
                            lhsT=yG8[:, 2 * j:2 * j + 2, mt * P:(mt + 1) * P],
                            rhs=wproj8[:, 2 * j:2 * j + 2, ch * 512:(ch + 1) * 512],
                            start=(j == 0), stop=(j == QH // 2 - 1),
                            perf_mode=DR)
                    if ch % 2 == 0:
                        nc.vector.tensor_scalar(out=ht[:, ch * 512:(ch + 1) * 512],
                                                in0=pp[:], scalar1=1.0 / WS,
                                                scalar2=None, op0=ALU.mult)
                    else:
                        nc.scalar.activation(out=ht[:, ch * 512:(ch + 1) * 512],
                                             in_=pp[:], func=AF.Identity,
                                             bias=0.0, scale=1.0 / WS)
            nc.sync.dma_start(out=hout_d[mt * P:(mt + 1) * P, :], in_=ht[:])

        with tc.tile_pool(name="pieces", bufs=6) as pieces_pool, \
             tc.tile_pool(name="pc8", bufs=16) as pc8_pool, \
             tc.tile_pool(name="rrep", bufs=2) as rrep_pool:

            cosA = pieces_pool.tile([32, 1536], BF16, tag="cosA", bufs=1, name="cosA")
            nc.sync.dma_start(out=cosA[:], in_=cosT_in[:, 512:T])
            sinA = pieces_pool.tile([32, 1536], BF16, tag="sinA", bufs=1, name="sinA")
            nc.sync.dma_start(out=sinA[:], in_=sinT_in[:, 512:T])

            def exp_diag(dst128, src128):
                """exp the 128-wide diagonal boundary block then tri-mask it."""
                scr = pieces_pool.tile([P, P], BF16, tag="scr", bufs=4)
                nc.scalar.activation(out=scr[:], in_=src128, func=AF.Exp,
                                     bias=eb_t[:], scale=1.0)
                nc.gpsimd.tensor_tensor(out=dst128, in0=scr[:], in1=tri[:],
                                        op=ALU.mult)

            def emit_scoresT(h, tg):
                nsb = 4 * tg + 4
                out_pieces = []
                for sb in range(nsb):
                    j = sb - 4 * tg     # >= 0 on diagonal pieces
                    lo = max(j, 0) * P  # masked-to-zero prefix width
                    ps_ = psMM.tile([P, 512], F32, tag="mm")
                    nc.tensor.matmul(ps_[:, lo:512],
                                     lhsT=qkvT[4][:, sb * P:(sb + 1) * P],
                                     rhs=qkvT[h][:, tg * 512 + lo:(tg + 1) * 512],
                                     start=True, stop=True)
                    if tg == 0:
                        pc = pieces_pool.tile([P, 512], BF16, tag="pcb", bufs=9)
                        if j > 0:
                            nc.gpsimd.memset(pc[:, 0:j * P], 0.0)
                        if j < 3:
                            nc.scalar.activation(out=pc[:, (j + 1) * P:512],
                                                 in_=ps_[:, (j + 1) * P:512],
                                                 func=AF.Exp, bias=eb_t[:], scale=1.0)
                        exp_diag(pc[:, j * P:(j + 1) * P], ps_[:, j * P:(j + 1) * P])
                        out_pieces.append(pc)
                    else:
                        if sb % 2 == 0:
                            pair = pc8_pool.tile([P, 2, 512], F8, tag="pc8", bufs=16)
                            out_pieces.append(pair)
                        else:
                            pair = out_pieces[-1]
                        if j < 0:
                            nc.scalar.activation(out=pair[:, sb % 2, :], in_=ps_[:],
                                                 func=AF.Exp, bias=eb_t[:], scale=1.0)
                        else:
                            if j > 0:
                                nc.gpsimd.memset(pair[:, sb % 2, 0:j * P], 0.0)
                            if j < 3:
                                nc.scalar.activation(out=pair[:, sb % 2, (j + 1) * P:512],
                                                     in_=ps_[:, (j + 1) * P:512],
                                                     func=AF.Exp, bias=eb_t[:], scale=1.0)
                            exp_diag(pair[:, sb % 2, j * P:(j + 1) * P],
                                     ps_[:, j * P:(j + 1) * P])
                return out_pieces

            def emit_pv(h, tg, pcs):
                psd = psD.tile([P, 512], F32, tag="psd")
                if tg == 0:
                    for sb, pc in enumerate(pcs):
                        nc.tensor.matmul(psd[:], lhsT=ones_bf[:], rhs=pc[:],
                                         start=(sb == 0), stop=(sb == len(pcs) - 1))
                else:
                    for j, pair in enumerate(pcs):
                        nc.tensor.matmul(psd[:], lhsT=ones8[:], rhs=pair[:],
                                         start=(j == 0), stop=(j == len(pcs) - 1),
                                         perf_mode=DR)
                rr = rrep_pool.tile([P, 512], F32, tag="rr")
                nc.vector.reciprocal(rr[:], psd[:])
                psy = psY.tile([P, 512], F32, tag="psy")
                if tg == 0:
                    for sb, pc in enumerate(pcs):
                        nc.tensor.matmul(psy[:], lhsT=vb[:, sb, :], rhs=pc[:],
                                         start=(sb == 0), stop=(sb == len(pcs) - 1))
                    nc.vector.tensor_tensor(out=yGb[h][:], in0=psy[:], in1=rr[:],
                                            op=ALU.mult)
                else:
                    for j, pair in enumerate(pcs):
                        nc.tensor.matmul(psy[:], lhsT=v_tok8[:, 2 * j:2 * j + 2, :],
                                         rhs=pair[:],
                                         start=(j == 0), stop=(j == len(pcs) - 1),
                                         perf_mode=DR)
                    nc.vector.tensor_tensor(out=yG8[:, h, tg * 512:(tg + 1) * 512],
                                            in0=psy[:], in1=rr[:], op=ALU.mult)

            window = deque()
            pops = [0]

            def pop_one():
                # interleave an fc1 block BEFORE the (stall-prone) pv chain
                if pops[0] % 2 == 1 and fc1_next[0] < NF - NF_B:
                    emit_fc1_block()
                pops[0] += 1
                ph, ptg, cur = window.popleft()
                emit_pv(ph, ptg, cur)
                if ph == QH - 1:
                    for mt in range(4 * ptg, 4 * ptg + 4):
                        emit_D(mt)

            fc1_dma()
            fc1_dma()
            fc1_dma()
            emit_fc1_block()
            emit_fc1_block()
            for tg in range(4):
                if tg >= 1:
                    # lazy rope for this chunk (q heads + k), off the stage-B
                    # critical path
                    co = slice((tg - 1) * 512, tg * 512)
                    for m in (4, 0, 1, 2, 3):
                        emit_rope(m, tg, pieces_pool, cosA[:, co], sinA[:, co])
                for h in range(QH):
                    window.append((h, tg, emit_scoresT(h, tg)))
                    if len(window) > 2:
                        pop_one()
            while window:
                pop_one()

        hsb_cm.__exit__(None, None, None)
        wproj_cm.__exit__(None, None, None)
        vtok_cm.__exit__(None, None, None)
        qkvT_cm.__exit__(None, None, None)
        yG_cm.__exit__(None, None, None)

        # ================= Stage E: fc1 remainder =========================
        uT_cm = tc.tile_pool(name="uT", bufs=1, side="right")
        uT_pool = uT_cm.__enter__()
        for i in range(NF_B):
            uTb.append(uT_pool.tile([P, T], BF16, tag="uTb", name=f"uTb{i}",
                                    bufs=NF_B))
        wfc2_cm = tc.tile_pool(name="wfc2", bufs=1, side="right")
        wfc2_pool = wfc2_cm.__enter__()
        wfc2b = []
        wfc28 = [None]
        ring = [(psMM, "mm"), (psMM, "mm"), (psY, "psy"), (psD, "psd")]
        while fc1_next[0] < NF:
            emit_fc1_block(ps_pools=ring)
            if fc1_next[0] == 12:
                # prefetch fc2 weights mid-fc1
                for k in range(NF_B):
                    wt = wfc2_pool.tile([P, C], BF16, tag="wfc2b", name=f"wfc2b{k}",
                                        bufs=NF_B)
                    nc.sync.dma_start(out=wt[:], in_=wfc2b_in[:, k, :])
                    wfc2b.append(wt)
                wfc28[0] = wfc2_pool.tile([P, NF - NF_B, C], F8, tag="wfc28", bufs=1, name="wfc28")
                nc.sync.dma_start(out=wfc28[0][:], in_=wfc28_in[:])

        # ================= Stage F: fc2 (mlp partial only) ================
        uball = uTb
        with tc.tile_pool(name="outsb", bufs=3) as outsb_pool:
            for mt in range(NT):
                ot = outsb_pool.tile([P, C], BF16, tag="ot")
                for ch in range(4):
                    pool, ptag = ring[(mt * 4 + ch) % 4]
                    pf = pool.tile([P, 512], F32, tag=ptag)
                    for k in range(NF_B):
                        nc.tensor.matmul(
                            pf[:],
                            lhsT=uball[k][:, mt * P:(mt + 1) * P],
                            rhs=wfc2b[k][:, ch * 512:(ch + 1) * 512],
                            start=(k == 0), stop=False)
                    for j in range((NF - NF_B) // 2):
                        nc.tensor.matmul(
                            pf[:],
                            lhsT=uT8[0][:, 2 * j:2 * j + 2, mt * P:(mt + 1) * P],
                            rhs=wfc28[0][:, 2 * j:2 * j + 2, ch * 512:(ch + 1) * 512],
                            start=False, stop=(j == (NF - NF_B) // 2 - 1),
                            perf_mode=DR)
                    if ch % 2 == 0:
                        nc.vector.tensor_scalar(out=ot[:, ch * 512:(ch + 1) * 512],
                                                in0=pf[:], scalar1=1.0 / WS,
                                                scalar2=None, op0=ALU.mult)
                    else:
                        nc.scalar.activation(out=ot[:, ch * 512:(ch + 1) * 512],
                                             in_=pf[:], func=AF.Identity,
                                             bias=0.0, scale=1.0 / WS)
                    nc.sync.dma_start(
                        out=out_d[mt * P:(mt + 1) * P, ch * 512:(ch + 1) * 512],
                        in_=ot[:, ch * 512:(ch + 1) * 512])

        psD_cm.__exit__(None, None, None)
        psY_cm.__exit__(None, None, None)
        psMM_cm.__exit__(None, None, None)
        wfc2_cm.__exit__(None, None, None)
        uT_cm.__exit__(None, None, None)
        uTa_cm.__exit__(None, None, None)
        wblk_cm.__exit__(None, None, None)
        xhatT_cm.__exit__(None, None, None)
        const_cm.__exit__(None, None, None)
        dram_cm.__exit__(None, None, None)

    _split_sync_waits(nc)
    return nc


def _sin_signed(sin):
    s = sin.T.copy()          # (32, T)
    s[0:16, :] *= -1.0        # rope: rotated = (-x2, x1); fold the minus into sin
    return s


def _prep_core_inputs(inputs, b, g):
    """Host-side slicing + LN-fold + fp8 quantization for core (b, g)."""
    x = np.asarray(inputs["x"], np.float32)
    cos = np.asarray(inputs["cos"], np.float32)
    sin = np.asarray(inputs["sin"], np.float32)
    ln1_w = np.asarray(inputs["ln1_w"], np.float32)
    ln1_b = np.asarray(inputs["ln1_b"], np.float32)
    ln2_w = np.asarray(inputs["ln2_w"], np.float32)
    ln2_b = np.asarray(inputs["ln2_b"], np.float32)
    w_qkv = np.asarray(inputs["w_qkv"], np.float32)
    b_qkv = np.asarray(inputs["b_qkv"], np.float32)
    w_proj = np.asarray(inputs["w_proj"], np.float32)
    w_fc1 = np.asarray(inputs["w_fc1"], np.float32)
    b_fc1 = np.asarray(inputs["b_fc1"], np.float32)
    w_fc2 = np.asarray(inputs["w_fc2"], np.float32)

    s = 1.0 / np.sqrt(np.float32(D))
    Wg = (w_qkv[:, g * GCOLS:(g + 1) * GCOLS] * ln1_w[:, None]).astype(np.float32)
    bg = (ln1_b @ w_qkv + b_qkv)[g * GCOLS:(g + 1) * GCOLS].astype(np.float32).copy()
    Wg = Wg.copy()
    Wg[:, :QH * D] *= s
    bg[:QH * D] *= s

    Wf1 = (w_fc1[:, g * FFN_S:(g + 1) * FFN_S] * ln2_w[:, None]).astype(np.float32)
    bf1 = (ln2_b @ w_fc1 + b_fc1)[g * FFN_S:(g + 1) * FFN_S].astype(np.float32)

    bv = bg[5 * P:6 * P]
    wproj_g = np.ascontiguousarray(w_proj[g * QH * D:(g + 1) * QH * D, :])
    bias_extra = np.tile(bv, QH) @ wproj_g
    wproj8 = (wproj_g * WS).reshape(QH, P, C).transpose(1, 0, 2)

    wfc2_g = np.ascontiguousarray(w_fc2[g * FFN_S:(g + 1) * FFN_S, :])
    # fc2 weights x64 so bf16 and fp8 halves share one psum accumulation
    wfc2b = (wfc2_g[:NF_B * P] * WS).reshape(NF_B, P, C).transpose(1, 0, 2)
    wfc28 = (wfc2_g[NF_B * P:] * WS).reshape(NF - NF_B, P, C).transpose(1, 0, 2)

    negSq = -Wg.astype(BF16NP).astype(np.float32).sum(0)
    negSf1 = -Wf1.astype(BF16NP).astype(np.float32).sum(0)

    return {
        "_bias_extra": bias_extra,
        "xT": np.ascontiguousarray(x[b].T).astype(BF16NP),
        "negSq": negSq.reshape(1, GCOLS).astype(BF16NP),
        "negSf1": negSf1.reshape(1, FFN_S).astype(BF16NP),
        "wqkv": Wg.astype(BF16NP),
        "bqkvT": np.ascontiguousarray(bg.reshape(6, P).T),
        "cosT": np.ascontiguousarray(cos.T).astype(BF16NP),
        "sinT": np.ascontiguousarray(_sin_signed(sin)).astype(BF16NP),
        "wprojb": wproj_g.astype(BF16NP),
        "wproj8": np.ascontiguousarray(wproj8).astype(F8NP),
        "wfc1": Wf1.astype(BF16NP),
        "bfc1T": np.ascontiguousarray(bf1.reshape(NF, P).T),
        "wfc2b": np.ascontiguousarray(wfc2b).astype(BF16NP),
        "wfc28": np.ascontiguousarray(wfc28).astype(F8NP),
    }


def kernel(**inputs):
    global _CACHED_NC
    if _CACHED_NC is None:
        _CACHED_NC = build_program()
    nc = _CACHED_NC

    B = inputs["x"].shape[0]
    in_maps = []
    bias_extra = np.zeros((C,), np.float32)
    for core in range(8):
        b, g = core // 4, core % 4
        m = _prep_core_inputs(inputs, b, g)
        if b == 0:
            bias_extra += m.pop("_bias_extra")
        else:
            m.pop("_bias_extra")
        in_maps.append(m)

    res = run_bass_kernel_spmd(nc, in_maps, core_ids=list(range(8)))

    b_proj = np.asarray(inputs["b_proj"], np.float32)
    b_fc2 = np.asarray(inputs["b_fc2"], np.float32)
    x = np.asarray(inputs["x"], np.float32)
    out = np.empty((B, T, C), np.float32)
    for b in range(B):
        acc = res.results[b * 4 + 0]["out"].astype(np.float32)
        acc += res.results[b * 4 + 0]["hout"].astype(np.float32)
        for g in range(1, 4):
            acc += res.results[b * 4 + g]["out"].astype(np.float32)
            acc += res.results[b * 4 + g]["hout"].astype(np.float32)
        out[b] = acc + x[b] + (b_proj + b_fc2 + bias_extra)[None, :]
    return out



# revision 54
# speedup vs baseline: 1.0759x; 1.0759x over previous
"""Trainium2 Bass kernel for a GQA transformer block (parallel-residual).

Reference computation (B=2, T=2048, C=2048, 16 heads / 4 query groups,
head_size=128, rope_n_elem=32, ffn=4C):
    qkv = LN1(x) @ w_qkv + b_qkv        (LN scale/bias folded into w/b host-side)
    q,k,v split per query group; RoPE on first 32 channels of q,k
    y   = causal_attention(q, k, v)
    h   = y @ w_proj + b_proj
    mlp = gelu(LN2(x) @ w_fc1 + b_fc1) @ w_fc2 + b_fc2
    out = mlp + h + x

Sharding: 8 cores = 2-way batch-parallel x 4-way tensor-parallel over query
groups.  Core c handles batch b=c//4, group g=c%4.  Each core emits two
partial outputs (mlp partial and h partial, both without output biases);
the host sums them plus x + biases.  No on-device collectives.

Design notes:
- The host passes x TRANSPOSED (feature-major, bf16).  LN stats (mean /
  variance per token) are computed with all-ones stationary matmuls
  (partition reduction on PE), producing partition-replicated [128,512]
  stat tiles; xhatT is then computed elementwise.  No PE transposes.
- fp8 (e4m3) DoubleRow matmuls (2x PE throughput) where numerics allow:
  attention PV + denominator for t>=512, out-projection for t>=512, and
  the upper half of the fc2 contraction.  Early tokens (t<512) stay
  bf16: with a small softmax support the fp8 quantization of probs/v/y
  creates outlier errors there.
- exp pieces are scaled by 1/16 (bias=-ln16) to fit fp8's 240 max; the
  softmax normalization cancels the scale exactly.
- Causal masking is trimmed: for a diagonal piece j only the 128-wide
  boundary block needs a mask multiply; columns below are memset to 0
  and columns above are exp'd directly.
- fc2 weights are scaled x64 (both halves) to share one PSUM group;
  evictions descale by 1/64.
- h is added on the host: fc2 writes the mlp partial to `out`, proj
  writes the h partial to `hout`.
- The first fc1 m-blocks are emitted interleaved into the attention
  group loop to fill PE gaps (the exp/DVE chain otherwise leaves the
  tensor engine idle long enough for the HAM clock throttle to drop it
  to half clock).
"""

import sys

sys.path.insert(0, "/opt/trn_rl_repo")

import math
import numpy as np
import ml_dtypes

import concourse.bass as bass
import concourse.mybir as mybir
import concourse.tile as tile
from concourse.bass_utils import run_bass_kernel_spmd

F32 = mybir.dt.float32
BF16 = mybir.dt.bfloat16
F8 = mybir.dt.float8e4
AF = mybir.ActivationFunctionType
ALU = mybir.AluOpType
DR = mybir.MatmulPerfMode.DoubleRow
BF16NP = ml_dtypes.bfloat16
F8NP = ml_dtypes.float8_e4m3

P = 128
T = 2048
C = 2048
D = 128
NT = T // P          # token tiles
NK = C // P          # contraction tiles over C
QH = 4               # query heads per group
GCOLS = (QH + 2) * D  # 768 qkv columns per group
FFN_S = 2048         # ffn shard per core (8192/4)
NF = FFN_S // P
NF_B = 8             # fc2 k-tiles computed in bf16 (0..7); 8..15 in fp8 DR
NF1_B = 12           # fc1 k-tiles computed in bf16 (0..11); 12..15 in fp8 DR
N_EARLY = 6          # fc1 m-blocks interleaved into the attention loop
LN_EPS = 1e-5
WS = 64.0            # fp8 weight scale (and bf16-fc2 scale for psum sharing)
EB = -math.log(16.0)  # exp bias: pieces = exp(score - ln 16)

_CACHED_NC = None


def _split_sync_waits(nc, limit=1):
    """This walrus build rejects instructions carrying more than one sem wait
    (setupSyncWait 'Too many sync wait commands'); move excess waits onto
    preceding NoOps on the same engine."""
    for f in nc.m.functions:
        for blk in f.blocks:
            new_list = []
            for inst in blk.instructions:
                si = inst.sync_info
                if si is not None and si.on_wait is not None and len(si.on_wait) > limit:
                    waits = list(si.on_wait)
                    head, rest = waits[:limit], waits[limit:]
                    k = 0
                    while rest:
                        chunk, rest = rest[:limit], rest[limit:]
                        new_list.append(
                            mybir.InstNoOp(
                                name=f"{inst.name}-ws{k}",
                                sync_info=mybir.SyncInfo(on_wait=chunk, on_update=[]),
                                bass_nofuse=True,
                                engine=inst.engine,
                            )
                        )
                        k += 1
                    inst.sync_info = mybir.SyncInfo(
                        on_wait=head, on_update=list(si.on_update or [])
                    )
                new_list.append(inst)
            blk.instructions[:] = new_list


def build_program():
    nc = bass.Bass()
    with tile.TileContext(nc) as tc:
        dram_cm = tc.tile_pool(name="dram", bufs=1, space="DRAM")
        dram = dram_cm.__enter__()
        # host pre-tiles x and wqkv so each loads with ONE batched DMA
        # (DMA_DIRECT2D issue is ~620ns serialized on the sync queue; 64
        # per-tile x DMAs took 40us just to issue)
        xr_in = dram.tile([P, NK, T], BF16, kind="ExternalInput", name="xr", uniquify=False)
        wqkv_in = dram.tile([P, NK, GCOLS], BF16, kind="ExternalInput", name="wqkv", uniquify=False)
        bqkvT_in = dram.tile([P, 6], F32, kind="ExternalInput", name="bqkvT", uniquify=False)
        # cos/sin tiled 4x along partitions for the packed q-head rope
        cosT_in = dram.tile([P, T], BF16, kind="ExternalInput", name="cosT", uniquify=False)
        sinT_in = dram.tile([P, T], BF16, kind="ExternalInput", name="sinT", uniquify=False)
        wprojb_in = dram.tile([QH * D, C], BF16, kind="ExternalInput", name="wprojb", uniquify=False)
        wproj8_in = dram.tile([P, QH, C], F8, kind="ExternalInput", name="wproj8", uniquify=False)
        # fc1 weights pre-tiled per m-block: [NF, P, NK, P] contiguous
        wfc1b_in = dram.tile([NF, P, NK, P], BF16, kind="ExternalInput", name="wfc1b", uniquify=False)
        bfc1T_in = dram.tile([P, NF], F32, kind="ExternalInput", name="bfc1T", uniquify=False)
        wfc2b_in = dram.tile([P, NF_B, C], BF16, kind="ExternalInput", name="wfc2b", uniquify=False)
        wfc28_in = dram.tile([P, NF - NF_B, C], F8, kind="ExternalInput", name="wfc28", uniquify=False)
        out_d = dram.tile([T, C], BF16, kind="ExternalOutput", name="out", uniquify=False)
        hout_d = dram.tile([T, C], BF16, kind="ExternalOutput", name="hout", uniquify=False)

        # ---- persistent pools ----
        const_cm = tc.tile_pool(name="const", bufs=1)
        const = const_cm.__enter__()
        # 128x128 lower-triangular boundary mask: 1 where t - s >= 0
        tri = const.tile([P, P], BF16, tag="tri")
        nc.gpsimd.memset(tri[:], 1.0)
        nc.gpsimd.affine_select(
            out=tri[:], in_=tri[:], compare_op=ALU.is_ge, fill=0.0,
            base=0, pattern=[[1, P]], channel_multiplier=-1)
        ones_bf = const.tile([P, P], BF16, tag="ones_bf")
        nc.vector.memset(ones_bf[:], 1.0)
        ones8 = const.tile([P, 2, P], F8, tag="ones8")
        nc.vector.memset(ones8[:], 1.0)
        bqkvT = const.tile([P, 6], F32, tag="bqkvT")
        bfc1T = const.tile([P, NF], F32, tag="bfc1T")
        eps_t = const.tile([P, 1], F32, tag="eps")
        nc.vector.memset(eps_t[:], LN_EPS)
        eb_t = const.tile([P, 1], F32, tag="eb")
        nc.vector.memset(eb_t[:], EB)

        def emit_const_dmas():
            nc.sync.dma_start(out=bqkvT[:], in_=bqkvT_in[:])
            nc.sync.dma_start(out=bfc1T[:], in_=bfc1T_in[:])

        xhatT_cm = tc.tile_pool(name="xhatT", bufs=NK)
        xhatT_pool = xhatT_cm.__enter__()
        xhatT = [xhatT_pool.tile([P, T], BF16, tag="xhatT", name=f"xhatT{i}") for i in range(NK)]

        # fc1 weight-block stream (lives through attention + fc1)
        wblk_cm = tc.tile_pool(name="wblk", bufs=1)
        wblk_pool = wblk_cm.__enter__()

        # shared matmul-accumulator psum pools
        psMM_cm = tc.tile_pool(name="psMM", bufs=4, space="PSUM")
        psMM = psMM_cm.__enter__()
        psY_cm = tc.tile_pool(name="psY", bufs=2, space="PSUM")
        psY = psY_cm.__enter__()
        psD_cm = tc.tile_pool(name="psD", bufs=2, space="PSUM")
        psD = psD_cm.__enter__()

        # ========== Stage A+B: LN stats + xhatT, then QKV ==================
        qkvT_cm = tc.tile_pool(name="qkvT", bufs=5)
        qkvT_pool = qkvT_cm.__enter__()
        qkvT = [qkvT_pool.tile([P, T], BF16, tag="qkvT", name=f"qkvT{i}") for i in range(5)]

        def emit_rope_k(nch, pool, ct, st):
            # k-head rope = x*cos + rot16(x)*sinT; sinT sign-folded by the
            # host; rot16 via partition-shifting SBUF->SBUF DMAs.  ct/st are
            # [32,512] cos/sin slices for this chunk.
            ch = slice(nch * 512, (nch + 1) * 512)
            rot = pool.tile([32, 512], BF16, tag="rot", bufs=2, name=f"rotk_{nch}")
            nc.sync.dma_start(out=rot[0:16, :], in_=qkvT[4][16:32, ch])
            nc.sync.dma_start(out=rot[16:32, :], in_=qkvT[4][0:16, ch])
            t_cos = pool.tile([32, 512], BF16, tag="t_cos", bufs=2, name=f"tck_{nch}")
            nc.vector.tensor_tensor(out=t_cos[:], in0=qkvT[4][0:32, ch],
                                    in1=ct, op=ALU.mult)
            nc.gpsimd.tensor_tensor(out=rot[:], in0=rot[:],
                                    in1=st, op=ALU.mult)
            nc.gpsimd.tensor_tensor(out=qkvT[4][0:32, ch], in0=t_cos[:],
                                    in1=rot[:], op=ALU.add)

        # DVE tensor_tensor time is free-dim driven (a [32,512] op costs the
        # same as [128,512]), so the 4 q-heads' rope rows are packed into one
        # [128,512] buffer (SBUF-SBUF DMAs on the idle sync engine), roped
        # with 3 full-width TTs, and DMA'd back -- 3 TTs/chunk instead of 12.
        rope4_cm = tc.tile_pool(name="rope4", bufs=1)
        rope4_pool = rope4_cm.__enter__()
        rope_x4 = [rope4_pool.tile([P, 512], BF16, tag="ropex4", bufs=4,
                                   name=f"ropex4_{c}") for c in range(4)]

        def emit_rope_q_pack(m, nch):
            # called right after head m's qkv eviction: stage its rope rows
            nc.sync.dma_start(out=rope_x4[nch][32 * m:32 * m + 32, :],
                              in_=qkvT[m][0:32, nch * 512:(nch + 1) * 512])

        def emit_rope_q(nch, pool, ct4, st4):
            ch = slice(nch * 512, (nch + 1) * 512)
            x4 = rope_x4[nch]
            rot = pool.tile([P, 512], BF16, tag="rot4", bufs=2, name=f"rot4_{nch}")
            for m in range(QH):
                nc.sync.dma_start(out=rot[32 * m:32 * m + 16, :],
                                  in_=qkvT[m][16:32, ch])
                nc.sync.dma_start(out=rot[32 * m + 16:32 * m + 32, :],
                                  in_=qkvT[m][0:16, ch])
            t_cos = pool.tile([P, 512], BF16, tag="t_cos4", bufs=2, name=f"tc4_{nch}")
            nc.vector.tensor_tensor(out=t_cos[:], in0=x4[:], in1=ct4, op=ALU.mult)
            nc.gpsimd.tensor_tensor(out=rot[:], in0=rot[:], in1=st4, op=ALU.mult)
            nc.gpsimd.tensor_tensor(out=x4[:], in0=t_cos[:], in1=rot[:], op=ALU.add)
            for m in range(QH):
                nc.sync.dma_start(out=qkvT[m][0:32, ch],
                                  in_=x4[32 * m:32 * m + 32, :])
        vtok_cm = tc.tile_pool(name="vtok", bufs=1)
        vtok_pool = vtok_cm.__enter__()
        v_tok8 = vtok_pool.tile([P, NK, P], F8, tag="vtok8")
        vb = vtok_pool.tile([P, 4, P], BF16, tag="vb")

        wqkv_cm = tc.tile_pool(name="wqkv", bufs=1)
        wqkv_pool = wqkv_cm.__enter__()

        # fc1 weight-block DMA machinery (blocks prefetched during stage B)
        FC1_ORDER = list(range(NF_B, NF)) + list(range(NF_B))
        fc1_dma_next = [0]
        fc1_next = [0]
        from collections import deque
        wb_q = deque()
        uTb = []

        def fc1_dma():
            i = fc1_dma_next[0]
            if i >= NF:
                return
            fc1_dma_next[0] = i + 1
            m = FC1_ORDER[i]
            wb = wblk_pool.tile([P, NK, P], BF16, tag="wb", bufs=3, name=f"wb{m}")
            nc.sync.dma_start(out=wb[:], in_=wfc1b_in[m])
            wb_q.append(wb)

        with tc.tile_pool(name="xio", bufs=16) as xio, \
             tc.tile_pool(name="stat", bufs=2) as stat:

            cos0 = xio.tile([P, 512], BF16, tag="cos0", bufs=1, name="cos0")
            sin0 = xio.tile([P, 512], BF16, tag="sin0", bufs=1, name="sin0")

            # batched x DMAs: one [P, NK, 512] transfer per 512-token chunk
            xq_c = []
            for c in range(4):
                xt = xio.tile([P, NK, 512], BF16, tag="xq", name=f"xq{c}", bufs=3)
                xq_c.append(xt)
            # chunk 0 in two halves so the first stats chain starts sooner
            nc.sync.dma_start(out=xq_c[0][:, 0:NK // 2, :], in_=xr_in[:, 0:NK // 2, 0:512])
            nc.sync.dma_start(out=xq_c[0][:, NK // 2:NK, :], in_=xr_in[:, NK // 2:NK, 0:512])
            nc.sync.dma_start(out=xq_c[1][:], in_=xr_in[:, :, 512:1024])
            emit_const_dmas()
            nc.sync.dma_start(out=cos0[:], in_=cosT_in[:, 0:512])
            nc.sync.dma_start(out=sin0[:], in_=sinT_in[:, 0:512])
            nc.sync.dma_start(out=xq_c[2][:], in_=xr_in[:, :, 1024:1536])
            wqkv_t = wqkv_pool.tile([P, NK, GCOLS], BF16, tag="wqkv", name="wqkvt")
            nc.sync.dma_start(out=wqkv_t[:], in_=wqkv_in[:])
            nc.sync.dma_start(out=xq_c[3][:], in_=xr_in[:, :, 1536:2048])

            # warmup burst: keep PE busy through the first x DMA so the HAM
            # clock gate reaches 8/8 before the real matmuls start
            wps = psMM.tile([P, 512], F32, tag="mm", name="warm")
            for w in range(64):
                nc.tensor.matmul(wps[:, 0:P], lhsT=ones_bf[:], rhs=ones_bf[:],
                                 start=(w == 0), stop=(w == 63))

            def emit_stats(nch):
                ch = slice(nch * 512, (nch + 1) * 512)
                xq = xq_c[nch]
                xsq = []
                for k in range(NK):
                    sq = xio.tile([P, 512], BF16, tag="xsq", name=f"xsq{nch}_{k}", bufs=6)
                    nc.scalar.activation(out=sq[:], in_=xq[:, k, :], func=AF.Square,
                                         bias=0.0, scale=1.0)
                    xsq.append(sq)
                psA = psY.tile([P, 512], F32, tag="psy")
                for k in range(NK):
                    nc.tensor.matmul(psA[:], lhsT=ones_bf[:], rhs=xq[:, k, :],
                                     start=(k == 0), stop=(k == NK - 1))
                psB = psD.tile([P, 512], F32, tag="psd")
                for k in range(NK):
                    nc.tensor.matmul(psB[:], lhsT=ones_bf[:], rhs=xsq[k][:],
                                     start=(k == 0), stop=(k == NK - 1))
                mean = stat.tile([P, 512], F32, tag="mean", bufs=1)
                nc.vector.tensor_scalar(out=mean[:], in0=psA[:], scalar1=1.0 / C,
                                        scalar2=None, op0=ALU.mult)
                var = stat.tile([P, 512], F32, tag="var", bufs=1)
                rstd = stat.tile([P, 512], F32, tag="rstd")
                # rstd doubles as the mean^2 scratch before it holds rstd
                nc.gpsimd.tensor_tensor(out=rstd[:], in0=mean[:], in1=mean[:], op=ALU.mult)
                nc.vector.tensor_scalar(out=var[:], in0=psB[:], scalar1=1.0 / C,
                                        scalar2=None, op0=ALU.mult)
                nc.gpsimd.tensor_tensor(out=var[:], in0=var[:], in1=rstd[:], op=ALU.subtract)
                nc.scalar.activation(out=rstd[:], in_=var[:], func=AF.Sqrt,
                                     bias=eps_t[:], scale=1.0)
                nc.vector.reciprocal(rstd[:], rstd[:])
                # bf16 copy of rstd: the xhat tensor_tensor then runs all-16-
                # bit at 2x DVE rate (f32 in1 would force 1x)
                rstd_b = stat.tile([P, 512], BF16, tag="rstdb", bufs=4)
                nc.vector.tensor_copy(rstd_b[:], rstd[:])
                # xhatT holds x*rstd; the -mu part is pre-folded into the
                # centered weight columns (colsum == 0)
                for k in range(NK):
                    eng = nc.vector if k % 2 == 0 else nc.gpsimd
                    eng.tensor_tensor(out=xhatT[k][:, ch], in0=xq[:, k, :], in1=rstd_b[:],
                                      op=ALU.mult)

            def emit_qkv(nch):
                ch = slice(nch * 512, (nch + 1) * 512)
                for m in (4, 0, 1, 2, 3, 5):
                    if m == 5:
                        # v computed token-major; bias folded into the host-side
                        # output bias (softmax weights sum to 1).
                        for ti in range(4 * nch, 4 * nch + 4):
                            pb = psMM.tile([P, 512], F32, tag="mm")
                            for k in range(NK):
                                nc.tensor.matmul(pb[:, 0:P],
                                                 lhsT=xhatT[k][:, ti * P:(ti + 1) * P],
                                                 rhs=wqkv_t[:, k, 5 * P:6 * P],
                                                 start=(k == 0), stop=(k == NK - 1))
                            nc.scalar.activation(out=v_tok8[:, ti, :], in_=pb[:, 0:P],
                                                 func=AF.Identity, bias=0.0, scale=1.0)
                            if ti < 4:
                                nc.scalar.copy(vb[:, ti, :], pb[:, 0:P])
                        continue
                    pb = psMM.tile([P, 512], F32, tag="mm")
                    for k in range(NK):
                        nc.tensor.matmul(pb[:], lhsT=wqkv_t[:, k, m * P:(m + 1) * P],
                                         rhs=xhatT[k][:, ch],
                                         start=(k == 0), stop=(k == NK - 1))
                    nc.scalar.activation(out=qkvT[m][:, ch],
                                         in_=pb[:], func=AF.Identity,
                                         bias=bqkvT[:, m:m + 1], scale=1.0)
                    if m != 4:
                        emit_rope_q_pack(m, nch)
                    if m == 3 and nch == 0:
                        emit_rope_q(0, xio, cos0[:], sin0[:])
                        emit_rope_k(0, xio, cos0[0:32, :], sin0[0:32, :])

            emit_stats(0)
            emit_stats(1)
            emit_stats(2)
            emit_stats(3)
            emit_qkv(0)
            fc1_dma()
            fc1_dma()
            emit_qkv(1)
            fc1_dma()
            emit_qkv(2)
            emit_qkv(3)

        wqkv_cm.__exit__(None, None, None)

        # ================= Stage C: causal attention ======================
        # (s, t) score layout; fc1 early blocks interleaved to keep PE warm.
        uTa_cm = tc.tile_pool(name="uTa", bufs=1, side="right")
        uTa_pool = uTa_cm.__enter__()
        uT8 = [uTa_pool.tile([P, NF - NF_B, T], F8, tag="uT8", bufs=1, name="uT8")]

        yG_cm = tc.tile_pool(name="yG", bufs=1, side="right")
        yG_pool = yG_cm.__enter__()
        yGb = [yG_pool.tile([P, 512], BF16, tag="yGb", name=f"yGb{i}", bufs=QH) for i in range(QH)]
        yG8 = yG_pool.tile([P, QH, T], F8, tag="yG8", bufs=1)

        wproj_cm = tc.tile_pool(name="wproj", bufs=1)
        wproj_pool = wproj_cm.__enter__()
        hsb_cm = tc.tile_pool(name="hsb", bufs=2)
        hsb_pool = hsb_cm.__enter__()
        wprojb = []
        for k in range(QH):
            wt = wproj_pool.tile([P, C], BF16, tag="wprojb", name=f"wprojb{k}", bufs=QH)
            nc.sync.dma_start(out=wt[:], in_=wprojb_in[k * P:(k + 1) * P, :])
            wprojb.append(wt)
        wproj8 = wproj_pool.tile([P, QH, C], F8, tag="wproj8", bufs=1)
        nc.sync.dma_start(out=wproj8[:], in_=wproj8_in[:])

        def emit_fc1_block(ps_pools=None):
            i = fc1_next[0]
            if i >= NF:
                return
            fc1_next[0] = i + 1
            m = FC1_ORDER[i]
            if not wb_q:
                fc1_dma()
            wb = wb_q.popleft()
            fc1_dma()
            for nch in range(4):
                if ps_pools is None:
                    pool, ptag = psMM, "mm"
                else:
                    pool, ptag = ps_pools[nch]
                ch = slice(nch * 512, (nch + 1) * 512)
                pe_ = pool.tile([P, 512], F32, tag=ptag)
                for k in range(NK):
                    nc.tensor.matmul(pe_[:], lhsT=wb[:, k, :],
                                     rhs=xhatT[k][:, ch],
                                     start=(k == 0), stop=(k == NK - 1))
                if m < NF_B:
                    dst = uTb[m][:, ch]
                else:
                    dst = uT8[0][:, m - NF_B, ch]
                nc.scalar.activation(out=dst, in_=pe_[:], func=AF.Gelu,
                                     bias=bfc1T[:, m:m + 1], scale=1.0)

        def emit_D(mt):
            ht = hsb_pool.tile([P, C], BF16, tag="ht")
            for ch in range(4):
                pp = psMM.tile([P, 512], F32, tag="mm")
                if mt < 4:
                    for k in range(QH):
                        nc.tensor.matmul(
                            pp[:],
                            lhsT=yGb[k][:, mt * P:(mt + 1) * P],
                            rhs=wprojb[k][:, ch * 512:(ch + 1) * 512],
                            start=(k == 0), stop=(k == QH - 1))
                    if ch % 2 == 0:
                        nc.vector.tensor_copy(ht[:, ch * 512:(ch + 1) * 512], pp[:])
                    else:
                        nc.scalar.copy(ht[:, ch * 512:(ch + 1) * 512], pp[:])
                else:
                    for j in range(QH // 2):
                        nc.tensor.matmul(
                            pp[:],
                            lhsT=yG8[:, 2 * j:2 * j + 2, mt * P:(mt + 1) * P],
                            rhs=wproj8[:, 2 * j:2 * j + 2, ch * 512:(ch + 1) * 512],
                            start=(j == 0), stop=(j == QH // 2 - 1),
                            perf_mode=DR)
                    if ch % 2 == 0:
                        nc.vector.tensor_scalar(out=ht[:, ch * 512:(ch + 1) * 512],
                                                in0=pp[:], scalar1=1.0 / WS,
                                                scalar2=None, op0=ALU.mult)
                    else:
                        nc.scalar.activation(out=ht[:, ch * 512:(ch + 1) * 512],
                                             in_=pp[:], func=AF.Identity,
                                             bias=0.0, scale=1.0 / WS)
            nc.sync.dma_start(out=hout_d[mt * P:(mt + 1) * P, :], in_=ht[:])

        with tc.tile_pool(name="pieces", bufs=6) as pieces_pool, \
             tc.tile_pool(name="pc8", bufs=13) as pc8_pool, \
             tc.tile_pool(name="rrep", bufs=1) as rrep_pool:

            def load_cs_chunk(c):
                # per-chunk cos/sin ring (a [P,1536] block would not fit)
                ct = pieces_pool.tile([P, 512], BF16, tag="cosA", bufs=2,
                                      name=f"cosA{c}")
                nc.sync.dma_start(out=ct[:], in_=cosT_in[:, c * 512:(c + 1) * 512])
                st = pieces_pool.tile([P, 512], BF16, tag="sinA", bufs=2,
                                      name=f"sinA{c}")
                nc.sync.dma_start(out=st[:], in_=sinT_in[:, c * 512:(c + 1) * 512])
                return ct, st

            def exp_diag(dst128, src128):
                """exp the 128-wide diagonal boundary block then tri-mask it."""
                scr = pieces_pool.tile([P, P], BF16, tag="scr", bufs=4)
                nc.scalar.activation(out=scr[:], in_=src128, func=AF.Exp,
                                     bias=eb_t[:], scale=1.0)
                nc.gpsimd.tensor_tensor(out=dst128, in0=scr[:], in1=tri[:],
                                        op=ALU.mult)

            def emit_scoresT(h, tg):
                # diagonal pairs FIRST: their tri-mask (gpsimd) is the
                # latest-finishing producer, so give it maximal lead time
                # before emit_pv consumes the pieces
                out_pieces = []
                if tg == 0:
                    sb_order = range(4)
                else:
                    sb_order = [4 * tg, 4 * tg + 1, 4 * tg + 2, 4 * tg + 3] + \
                               list(range(0, 4 * tg))
                for sb in sb_order:
                    j = sb - 4 * tg     # >= 0 on diagonal pieces
                    lo = max(j, 0) * P  # masked-to-zero prefix width
                    ps_ = psMM.tile([P, 512], F32, tag="mm")
                    nc.tensor.matmul(ps_[:, lo:512],
                                     lhsT=qkvT[4][:, sb * P:(sb + 1) * P],
                                     rhs=qkvT[h][:, tg * 512 + lo:(tg + 1) * 512],
                                     start=True, stop=True)
                    if tg == 0:
                        pc = pieces_pool.tile([P, 512], BF16, tag="pcb", bufs=9)
                        if j > 0:
                            nc.gpsimd.memset(pc[:, 0:j * P], 0.0)
                        if j < 3:
                            nc.scalar.activation(out=pc[:, (j + 1) * P:512],
                                                 in_=ps_[:, (j + 1) * P:512],
                                                 func=AF.Exp, bias=eb_t[:], scale=1.0)
                        exp_diag(pc[:, j * P:(j + 1) * P], ps_[:, j * P:(j + 1) * P])
                        out_pieces.append((sb, pc))
                        continue
                    if sb % 2 == 0:
                        pair = pc8_pool.tile([P, 2, 512], F8, tag="pc8", bufs=13)
                        out_pieces.append((sb, pair))
                    else:
                        pair = out_pieces[-1][1]
                    if j < 0:
                        nc.scalar.activation(out=pair[:, sb % 2, :], in_=ps_[:],
                                             func=AF.Exp, bias=eb_t[:], scale=1.0)
                    else:
                        if j > 0:
                            nc.gpsimd.memset(pair[:, sb % 2, 0:j * P], 0.0)
                        if j < 3:
                            nc.scalar.activation(out=pair[:, sb % 2, (j + 1) * P:512],
                                                 in_=ps_[:, (j + 1) * P:512],
                                                 func=AF.Exp, bias=eb_t[:], scale=1.0)
                        exp_diag(pair[:, sb % 2, j * P:(j + 1) * P],
                                 ps_[:, j * P:(j + 1) * P])
                return out_pieces

            def emit_pv(h, tg, pcs):
                psd = psD.tile([P, 512], F32, tag="psd")
                if tg == 0:
                    for i, (sb, pc) in enumerate(pcs):
                        nc.tensor.matmul(psd[:], lhsT=ones_bf[:], rhs=pc[:],
                                         start=(i == 0), stop=(i == len(pcs) - 1))
                else:
                    for i, (sb, pair) in enumerate(pcs):
                        nc.tensor.matmul(psd[:], lhsT=ones8[:], rhs=pair[:],
                                         start=(i == 0), stop=(i == len(pcs) - 1),
                                         perf_mode=DR)
                rr = rrep_pool.tile([P, 512], F32, tag="rr")
                nc.vector.reciprocal(rr[:], psd[:])
                psy = psY.tile([P, 512], F32, tag="psy")
                if tg == 0:
                    for i, (sb, pc) in enumerate(pcs):
                        nc.tensor.matmul(psy[:], lhsT=vb[:, sb, :], rhs=pc[:],
                                         start=(i == 0), stop=(i == len(pcs) - 1))
                    nc.vector.tensor_tensor(out=yGb[h][:], in0=psy[:], in1=rr[:],
                                            op=ALU.mult)
                else:
                    for i, (sb, pair) in enumerate(pcs):
                        nc.tensor.matmul(psy[:], lhsT=v_tok8[:, sb:sb + 2, :],
                                         rhs=pair[:],
                                         start=(i == 0), stop=(i == len(pcs) - 1),
                                         perf_mode=DR)
                    nc.vector.tensor_tensor(out=yG8[:, h, tg * 512:(tg + 1) * 512],
                                            in0=psy[:], in1=rr[:], op=ALU.mult)

            window = deque()
            pops = [0]

            def pop_one():
                # interleave fc1 blocks in bursts of 2 BEFORE the (stall-
                # prone) pv chain; bursts (vs every-other-pop singles) halve
                # the exp<->gelu activation-table swaps on the scalar engine
                if pops[0] % 4 == 1 and fc1_next[0] < NF - NF_B:
                    emit_fc1_block()
                    emit_fc1_block()
                pops[0] += 1
                ph, ptg, cur = window.popleft()
                emit_pv(ph, ptg, cur)
                if ph == QH - 1:
                    for mt in range(4 * ptg, 4 * ptg + 4):
                        emit_D(mt)

            emit_fc1_block()
            emit_fc1_block()
            for tg in range(4):
                if tg < 3:
                    # lazy rope for the NEXT chunk, a full tg ahead of its
                    # first consumer so the DMA->TT->DMA latency chain hides
                    ct, st = load_cs_chunk(tg + 1)
                    emit_rope_q(tg + 1, pieces_pool, ct[:], st[:])
                    emit_rope_k(tg + 1, pieces_pool, ct[0:32, :], st[0:32, :])
                for h in range(QH):
                    window.append((h, tg, emit_scoresT(h, tg)))
                    if len(window) > 2:
                        pop_one()
            while window:
                pop_one()

        hsb_cm.__exit__(None, None, None)
        wproj_cm.__exit__(None, None, None)
        vtok_cm.__exit__(None, None, None)
        rope4_cm.__exit__(None, None, None)
        qkvT_cm.__exit__(None, None, None)
        yG_cm.__exit__(None, None, None)

        # ================= Stage E: fc1 remainder =========================
        uT_cm = tc.tile_pool(name="uT", bufs=1, side="right")
        uT_pool = uT_cm.__enter__()
        for i in range(NF_B):
            uTb.append(uT_pool.tile([P, T], BF16, tag="uTb", name=f"uTb{i}",
                                    bufs=NF_B))
        wfc2_cm = tc.tile_pool(name="wfc2", bufs=1, side="right")
        wfc2_pool = wfc2_cm.__enter__()
        wfc2b = [None]
        wfc28 = [None]
        ring = [(psMM, "mm"), (psMM, "mm"), (psY, "psy"), (psD, "psd")]
        while fc1_next[0] < NF:
            emit_fc1_block(ps_pools=ring)
            if fc1_next[0] == 9:
                # prefetch fc2 weights as early in the remainder as possible
                # (12MB; the DMA needs most of the fc1-remainder span).  The
                # fp8 half (4MB) first: the first fc2 rows consume it first.
                wfc28[0] = wfc2_pool.tile([P, NF - NF_B, C], F8, tag="wfc28", bufs=1, name="wfc28")
                nc.sync.dma_start(out=wfc28[0][:], in_=wfc28_in[:])
                wfc2b[0] = wfc2_pool.tile([P, NF_B, C], BF16, tag="wfc2b",
                                          bufs=1, name="wfc2b")
                nc.sync.dma_start(out=wfc2b[0][:], in_=wfc2b_in[:])

        # ================= Stage F: fc2 (mlp partial only) ================
        uball = uTb
        with tc.tile_pool(name="outsb", bufs=3) as outsb_pool:
            for mt in range(NT):
                ot = outsb_pool.tile([P, C], BF16, tag="ot")
                for ch in range(4):
                    pool, ptag = ring[(mt * 4 + ch) % 4]
                    pf = pool.tile([P, 512], F32, tag=ptag)
                    # first rows run the DR (fp8) half first -- those weights
                    # arrive first from the prefetch
                    dr_first = mt < 2
                    if dr_first:
                        for j in range((NF - NF_B) // 2):
                            nc.tensor.matmul(
                                pf[:],
                                lhsT=uT8[0][:, 2 * j:2 * j + 2, mt * P:(mt + 1) * P],
                                rhs=wfc28[0][:, 2 * j:2 * j + 2, ch * 512:(ch + 1) * 512],
                                start=(j == 0), stop=False,
                                perf_mode=DR)
                    for k in range(NF_B):
                        nc.tensor.matmul(
                            pf[:],
                            lhsT=uball[k][:, mt * P:(mt + 1) * P],
                            rhs=wfc2b[0][:, k, ch * 512:(ch + 1) * 512],
                            start=(not dr_first and k == 0),
                            stop=(dr_first and k == NF_B - 1))
                    if not dr_first:
                        for j in range((NF - NF_B) // 2):
                            nc.tensor.matmul(
                                pf[:],
                                lhsT=uT8[0][:, 2 * j:2 * j + 2, mt * P:(mt + 1) * P],
                                rhs=wfc28[0][:, 2 * j:2 * j + 2, ch * 512:(ch + 1) * 512],
                                start=False, stop=(j == (NF - NF_B) // 2 - 1),
                                perf_mode=DR)
                    if ch % 2 == 0:
                        nc.vector.tensor_scalar(out=ot[:, ch * 512:(ch + 1) * 512],
                                                in0=pf[:], scalar1=1.0 / WS,
                                                scalar2=None, op0=ALU.mult)
                    else:
                        nc.scalar.activation(out=ot[:, ch * 512:(ch + 1) * 512],
                                             in_=pf[:], func=AF.Identity,
                                             bias=0.0, scale=1.0 / WS)
                    nc.sync.dma_start(
                        out=out_d[mt * P:(mt + 1) * P, ch * 512:(ch + 1) * 512],
                        in_=ot[:, ch * 512:(ch + 1) * 512])

        psD_cm.__exit__(None, None, None)
        psY_cm.__exit__(None, None, None)
        psMM_cm.__exit__(None, None, None)
        wfc2_cm.__exit__(None, None, None)
        uT_cm.__exit__(None, None, None)
        uTa_cm.__exit__(None, None, None)
        wblk_cm.__exit__(None, None, None)
        xhatT_cm.__exit__(None, None, None)
        const_cm.__exit__(None, None, None)
        dram_cm.__exit__(None, None, None)

    _split_sync_waits(nc)
    return nc


def _sin_signed(sin):
    s = sin.T.copy()          # (32, T)
    s[0:16, :] *= -1.0        # rope: rotated = (-x2, x1); fold the minus into sin
    return s


def _prep_core_inputs(inputs, b, g):
    """Host-side slicing + LN-fold + fp8 quantization for core (b, g)."""
    x = np.asarray(inputs["x"], np.float32)
    cos = np.asarray(inputs["cos"], np.float32)
    sin = np.asarray(inputs["sin"], np.float32)
    ln1_w = np.asarray(inputs["ln1_w"], np.float32)
    ln1_b = np.asarray(inputs["ln1_b"], np.float32)
    ln2_w = np.asarray(inputs["ln2_w"], np.float32)
    ln2_b = np.asarray(inputs["ln2_b"], np.float32)
    w_qkv = np.asarray(inputs["w_qkv"], np.float32)
    b_qkv = np.asarray(inputs["b_qkv"], np.float32)
    w_proj = np.asarray(inputs["w_proj"], np.float32)
    w_fc1 = np.asarray(inputs["w_fc1"], np.float32)
    b_fc1 = np.asarray(inputs["b_fc1"], np.float32)
    w_fc2 = np.asarray(inputs["w_fc2"], np.float32)

    s = 1.0 / np.sqrt(np.float32(D))
    Wg = (w_qkv[:, g * GCOLS:(g + 1) * GCOLS] * ln1_w[:, None]).astype(np.float32)
    bg = (ln1_b @ w_qkv + b_qkv)[g * GCOLS:(g + 1) * GCOLS].astype(np.float32).copy()
    Wg = Wg.copy()
    Wg[:, :QH * D] *= s
    bg[:QH * D] *= s

    Wf1 = (w_fc1[:, g * FFN_S:(g + 1) * FFN_S] * ln2_w[:, None]).astype(np.float32)
    bf1 = (ln2_b @ w_fc1 + b_fc1)[g * FFN_S:(g + 1) * FFN_S].astype(np.float32)

    def center_bf16(W):
        # LN mean-subtraction folded into the weights: with colsum(W) == 0,
        # (x - mu) @ W == x @ W exactly, so the device skips the rank-1
        # -mu*rstd correction.  Row 0 absorbs the bf16 rounding residual.
        Wc = W - W.mean(0, keepdims=True)
        Wr = Wc.astype(BF16NP)
        r0 = (Wr[0].astype(np.float32)
              - Wr.astype(np.float32).sum(0)).astype(BF16NP)
        Wr = Wr.copy()
        Wr[0] = r0
        return Wr

    Wgc = center_bf16(Wg)          # (C, GCOLS) bf16
    Wf1c = center_bf16(Wf1)        # (C, FFN_S) bf16

    bv = bg[5 * P:6 * P]
    wproj_g = np.ascontiguousarray(w_proj[g * QH * D:(g + 1) * QH * D, :])
    bias_extra = np.tile(bv, QH) @ wproj_g
    wproj8 = (wproj_g * WS).reshape(QH, P, C).transpose(1, 0, 2)

    wfc2_g = np.ascontiguousarray(w_fc2[g * FFN_S:(g + 1) * FFN_S, :])
    # fc2 weights x64 so bf16 and fp8 halves share one psum accumulation
    wfc2b = (wfc2_g[:NF_B * P] * WS).reshape(NF_B, P, C).transpose(1, 0, 2)
    wfc28 = (wfc2_g[NF_B * P:] * WS).reshape(NF - NF_B, P, C).transpose(1, 0, 2)

    return {
        "_bias_extra": bias_extra,
        "xr": np.ascontiguousarray(
            x[b].T.astype(BF16NP).reshape(NK, P, T).transpose(1, 0, 2)),
        "wqkv": np.ascontiguousarray(
            Wgc.reshape(NK, P, GCOLS).transpose(1, 0, 2)),
        "bqkvT": np.ascontiguousarray(bg.reshape(6, P).T),
        "cosT": np.ascontiguousarray(np.tile(cos.T, (4, 1))).astype(BF16NP),
        "sinT": np.ascontiguousarray(np.tile(_sin_signed(sin), (4, 1))).astype(BF16NP),
        "wprojb": wproj_g.astype(BF16NP),
        "wproj8": np.ascontiguousarray(wproj8).astype(F8NP),
        "wfc1b": np.ascontiguousarray(
            Wf1c.reshape(NK, P, NF, P).transpose(2, 1, 0, 3)),
        "bfc1T": np.ascontiguousarray(bf1.reshape(NF, P).T),
        "wfc2b": np.ascontiguousarray(wfc2b).astype(BF16NP),
        "wfc28": np.ascontiguousarray(wfc28).astype(F8NP),
    }


def kernel(**inputs):
    global _CACHED_NC
    if _CACHED_NC is None:
        _CACHED_NC = build_program()
    nc = _CACHED_NC

    B = inputs["x"].shape[0]
    in_maps = []
    bias_extra = np.zeros((C,), np.float32)
    for core in range(8):
        b, g = core // 4, core % 4
        m = _prep_core_inputs(inputs, b, g)
        if b == 0:
            bias_extra += m.pop("_bias_extra")
        else:
            m.pop("_bias_extra")
        in_maps.append(m)

    res = run_bass_kernel_spmd(nc, in_maps, core_ids=list(range(8)))

    b_proj = np.asarray(inputs["b_proj"], np.float32)
    b_fc2 = np.asarray(inputs["b_fc2"], np.float32)
    x = np.asarray(inputs["x"], np.float32)
    out = np.empty((B, T, C), np.float32)
    for b in range(B):
        acc = res.results[b * 4 + 0]["out"].astype(np.float32)
        acc += res.results[b * 4 + 0]["hout"].astype(np.float32)
        for g in range(1, 4):
            acc += res.results[b * 4 + g]["out"].astype(np.float32)
            acc += res.results[b * 4 + g]["hout"].astype(np.float32)
        out[b] = acc + x[b] + (b_proj + b_fc2 + bias_extra)[None, :]
    return out

